# revision 1
# baseline (speedup 1.0000x reference)
"""nn_BaseGraphEncoder GIN message-passing kernel for 8 Trainium2 NeuronCores.

Self-contained: host-side sharding prep (numpy) + Bass/Tile device kernel.
See _DOC below for the strategy summary.
"""
_DOC = """
  - dst-shard nodes across 8 cores (6250/core, padded to 6272 = 49 windows x 128,
    windows balanced by in-degree).
  - per-layer bf16 node-feature table [50176, 300] replicated in each core's DRAM
    via AllGather; per-edge gather via indirect DMA (128 rows/call, int32 ids).
  - aggregation: per 128-edge tile, one-hot dst matrix (DVE is_equal vs iota),
    PE matmul S^T @ msgs accumulated in PSUM per 128-dst window.
  - MLP: feature-major (transposed) activations, weight-stationary matmuls,
    fp32 PSUM, ACT fused bias+relu; PE transposes at orientation flips.
  - layer 4: pooling pushed before W2 (segsum(h5) = segsum(t5) @ W2 + cnt*b2);
    one-hot pooling matmuls with appended mask column for counts; AllReduce of
    [256,769] partials; fold W2; LayerNorm; identical output on every core.
"""
import sys

sys.path.insert(0, "/opt/trn_rl_repo")
import numpy as np
import concourse.bass as bass
import concourse.bacc as bacc
import concourse.tile as tile
import concourse.mybir as mybir

F32 = mybir.dt.float32
BF16 = mybir.dt.bfloat16
I32 = mybir.dt.int32
AF = mybir.ActivationFunctionType
OP = mybir.AluOpType
NPBF16 = mybir.dt.np(BF16)

LN_EPS = 1e-5


class Cfg:
    def __init__(self, N, E, G, T_half, F=128, HID=300, NOUT=768, NC=8, L=5):
        self.N, self.E, self.G, self.F, self.HID, self.NOUT, self.NC, self.L = (
            N, E, G, F, HID, NOUT, NC, L)
        assert N % NC == 0
        self.SH = N // NC
        self.WPC = (self.SH + 127) // 128
        self.SHP = self.WPC * 128
        self.NP = self.SHP * NC
        self.T = T_half                     # tiles per window
        self.DPAD = HID
        self.CH = (HID + 127) // 128
        self.CHO = NOUT // 128
        self.G_CH = (G + 127) // 128
        self.TILES = self.WPC * self.T
        self.WGS = [list(range(w, min(w + 2, self.WPC))) for w in range(0, self.WPC, 2)]


def prep_inputs(x, edge_index, batch, params, cfg):
    c = cfg
    x = np.asarray(x, np.float32)
    src = np.asarray(edge_index[0], np.int64)
    dst = np.asarray(edge_index[1], np.int64)
    batch = np.asarray(batch, np.int64)

    deg = np.bincount(dst, minlength=c.N)

    slot_of = np.empty(c.N, np.int64)
    import heapq
    for core in range(c.NC):
        nodes = np.arange(core * c.SH, (core + 1) * c.SH)
        order = nodes[np.argsort(-deg[nodes], kind="stable")]
        heap = [(0, 0, w) for w in range(c.WPC)]
        heapq.heapify(heap)
        fill = np.zeros(c.WPC, np.int64)
        for n in order:
            while True:
                load, cnt, w = heapq.heappop(heap)
                if fill[w] < 128:
                    break
            slot_of[n] = w * 128 + fill[w]
            fill[w] += 1
            heapq.heappush(heap, (load + deg[n], int(fill[w]), w))
    pgid = (np.arange(c.N) // c.SH) * c.SHP + slot_of

    ecore = dst // c.SH
    ew = slot_of[dst] // 128
    epos = slot_of[dst] % 128
    esrc = pgid[src]

    idx_arr = np.zeros((c.NC, 128, c.TILES), np.int32)
    dsel_arr = np.full((c.NC, 128, c.TILES), 255.0, np.float32)

    maxcnt = 0
    for core in range(c.NC):
        em = ecore == core
        order = np.lexsort((epos[em], ew[em]))
        cw, cp, cs = ew[em][order], epos[em][order], esrc[em][order]
        for w in range(c.WPC):
            m = cw == w
            cnt = int(m.sum())
            maxcnt = max(maxcnt, cnt)
            if cnt > c.T * 128:
                raise RuntimeError(f"tile overflow: {cnt} > {c.T*128}")
            g0 = w * c.T
            j = np.arange(cnt)
            gt = g0 + j // 128
            lane = j % 128
            idx_arr[core, lane, gt] = cs[m]
            dsel_arr[core, lane, gt] = cp[m]

    x_table = np.zeros((c.NP, c.F), np.float32)
    x_table[pgid] = x
    x_local = np.zeros((c.NC, c.SHP, c.F), np.float32)
    for core in range(c.NC):
        nodes = np.arange(core * c.SH, (core + 1) * c.SH)
        x_local[core, slot_of[nodes]] = x[nodes]

    bgs = np.full((c.NC, c.G_CH, 128, c.WPC), 300.0, np.float32)
    maskw = np.zeros((c.NC, 128, c.WPC), np.float32)
    for core in range(c.NC):
        nodes = np.arange(core * c.SH, (core + 1) * c.SH)
        sl = slot_of[nodes]
        w, p = sl // 128, sl % 128
        maskw[core, p, w] = 1.0
        for gc in range(c.G_CH):
            val = batch[nodes] - gc * 128
            ok = (val >= 0) & (val < 128)
            bgs[core, gc, p[ok], w[ok]] = val[ok]

    def padw(W, rpad, cpad):
        W = np.asarray(W, np.float32)
        out = np.zeros((rpad, cpad), np.float32)
        out[: W.shape[0], : W.shape[1]] = W
        return out.astype(NPBF16)

    wts = {}
    layers = params["layers"]
    for i in range(c.L):
        d_in = c.F if i == 0 else c.HID
        d_out = c.NOUT if i == c.L - 1 else c.HID
        ch_in = (d_in + 127) // 128
        ch_h = (d_out + 127) // 128
        dop = ch_h * 128
        wts[f"W1_{i}"] = padw(layers[i]["W1"], ch_in * 128, dop)
        wts[f"W2_{i}"] = padw(layers[i]["W2"], ch_h * 128, dop)
        b1 = np.zeros(ch_h * 128, np.float32); b1[:d_out] = np.asarray(layers[i]["b1"])
        b2 = np.zeros(ch_h * 128, np.float32); b2[:d_out] = np.asarray(layers[i]["b2"])
        if i < c.L - 1:
            wts[f"b1_{i}"] = b1.reshape(ch_h, 128).T.copy()
            wts[f"b2_{i}"] = b2.reshape(ch_h, 128).T.copy()
        else:
            wts[f"b1_{i}"] = np.tile(b1[None, : c.NOUT], (128, 1))
            wts[f"b2_{i}"] = np.tile(b2[None, : c.NOUT], (128, 1))
    wts["ln_w"] = np.tile(np.asarray(params["ln_w"], np.float32)[None, :], (128, 1))
    wts["ln_b"] = np.tile(np.asarray(params["ln_b"], np.float32)[None, :], (128, 1))

    in_maps = []
    for core in range(c.NC):
        m = {
            "x_table": x_table,
            "x_local": x_local[core],
            "idxs": idx_arr[core],
            "dsel": dsel_arr[core].astype(NPBF16),
            "bg": bgs[core].reshape(c.G_CH * 128, c.WPC).astype(NPBF16),
            "maskw": maskw[core].astype(NPBF16),
        }
        m.update(wts)
        in_maps.append(m)
    return in_maps, maxcnt


def build(cfg):
    c = cfg
    nc = bacc.Bacc("TRN2", target_bir_lowering=False, debug=False, num_devices=c.NC)
    RG = [list(range(c.NC))]

    x_table = nc.dram_tensor("x_table", [c.NP, c.F], F32, kind="ExternalInput")
    x_local = nc.dram_tensor("x_local", [c.SHP, c.F], F32, kind="ExternalInput")
    idxs_d = nc.dram_tensor("idxs", [128, c.TILES], I32, kind="ExternalInput")
    dsel_d = nc.dram_tensor("dsel", [128, c.TILES], BF16, kind="ExternalInput")
    bg_d = nc.dram_tensor("bg", [c.G_CH * 128, c.WPC], BF16, kind="ExternalInput")
    maskw_d = nc.dram_tensor("maskw", [128, c.WPC], BF16, kind="ExternalInput")
    wd = {}
    for i in range(c.L):
        d_in = c.F if i == 0 else c.HID
        d_out = c.NOUT if i == c.L - 1 else c.HID
        ch_in = (d_in + 127) // 128
        ch_h = (d_out + 127) // 128
        dop = ch_h * 128
        wd[f"W1_{i}"] = nc.dram_tensor(f"W1_{i}", [ch_in * 128, dop], BF16, kind="ExternalInput")
        wd[f"W2_{i}"] = nc.dram_tensor(f"W2_{i}", [ch_h * 128, dop], BF16, kind="ExternalInput")
        bshape = [128, ch_h] if i < c.L - 1 else [128, c.NOUT]
        wd[f"b1_{i}"] = nc.dram_tensor(f"b1_{i}", bshape, F32, kind="ExternalInput")
        wd[f"b2_{i}"] = nc.dram_tensor(f"b2_{i}", bshape, F32, kind="ExternalInput")
    ln_w_d = nc.dram_tensor("ln_w", [128, c.NOUT], F32, kind="ExternalInput")
    ln_b_d = nc.dram_tensor("ln_b", [128, c.NOUT], F32, kind="ExternalInput")
    out_d = nc.dram_tensor("out", [c.G, c.NOUT], F32, kind="ExternalOutput")

    with tile.TileContext(nc) as tc:
        from concourse.masks import make_identity
        import contextlib
        st = contextlib.ExitStack()
        with st:
            const = st.enter_context(tc.tile_pool(name="const", bufs=1))
            wpool = st.enter_context(tc.tile_pool(name="wpool", bufs=1))
            edgep = st.enter_context(tc.tile_pool(name="edgep", bufs=1))
            dram = st.enter_context(tc.tile_pool(name="dram", bufs=1, space="DRAM"))
            gpool = st.enter_context(tc.tile_pool(name="gpool", bufs=3))
            spool = st.enter_context(tc.tile_pool(name="spool", bufs=6))
            mpool = st.enter_context(tc.tile_pool(name="mpool", bufs=3))
            tpool = st.enter_context(tc.tile_pool(name="tpool", bufs=2))
            apsum = st.enter_context(tc.tile_pool(name="apsum", bufs=1, space="PSUM"))
            tpsum = st.enter_context(tc.tile_pool(name="tpsum", bufs=1, space="PSUM"))
            mlpps = st.enter_context(tc.tile_pool(name="mlpps", bufs=1, space="PSUM"))
            ppsum = st.enter_context(tc.tile_pool(name="ppsum", bufs=1, space="PSUM"))
            respool = st.enter_context(tc.tile_pool(name="respool", bufs=3))
            fpool = st.enter_context(tc.tile_pool(name="fpool", bufs=1))

            id_f32 = const.tile([128, 128], F32)
            make_identity(nc, id_f32[:])
            id_bf = const.tile([128, 128], BF16)
            nc.vector.tensor_copy(out=id_bf[:], in_=id_f32[:])
            iota32 = const.tile([128, 128], I32)
            nc.gpsimd.iota(iota32[:], pattern=[[1, 128]], base=0, channel_multiplier=0)
            iota_bf = const.tile([128, 128], BF16)
            nc.vector.tensor_copy(out=iota_bf[:], in_=iota32[:])
            iota_f32 = const.tile([128, 128], F32)
            nc.vector.tensor_copy(out=iota_f32[:], in_=iota32[:])
            eps_sb = const.tile([128, 1], F32)
            nc.vector.memset(eps_sb[:], LN_EPS)

            idx_sb = edgep.tile([128, c.TILES], I32)
            nc.sync.dma_start(out=idx_sb[:], in_=idxs_d[:])
            dsel_sb = edgep.tile([128, c.TILES], BF16)
            nc.sync.dma_start(out=dsel_sb[:], in_=dsel_d[:])
            bg_sb = edgep.tile([128, c.G_CH * c.WPC], BF16)
            for gc in range(c.G_CH):
                nc.sync.dma_start(out=bg_sb[:, gc * c.WPC:(gc + 1) * c.WPC],
                                  in_=bg_d[gc * 128:(gc + 1) * 128, :])
            mask_sb = edgep.tile([128, c.WPC], BF16)
            nc.sync.dma_start(out=mask_sb[:], in_=maskw_d[:])

            wsb = {}
            for i in range(c.L):
                d_in = c.F if i == 0 else c.HID
                d_out = c.NOUT if i == c.L - 1 else c.HID
                ch_in = (d_in + 127) // 128
                ch_h = (d_out + 127) // 128
                dop = ch_h * 128
                w1 = wpool.tile([128, ch_in * dop], BF16, tag=f"w1_{i}")
                for ki in range(ch_in):
                    nc.sync.dma_start(out=w1[:, ki * dop:(ki + 1) * dop],
                                      in_=wd[f"W1_{i}"][ki * 128:(ki + 1) * 128, :])
                w2 = wpool.tile([128, ch_h * dop], BF16, tag=f"w2_{i}")
                for ki in range(ch_h):
                    nc.sync.dma_start(out=w2[:, ki * dop:(ki + 1) * dop],
                                      in_=wd[f"W2_{i}"][ki * 128:(ki + 1) * 128, :])
                bshape = [128, ch_h] if i < c.L - 1 else [128, c.NOUT]
                b1 = wpool.tile(bshape, F32, tag=f"b1_{i}")
                nc.sync.dma_start(out=b1[:], in_=wd[f"b1_{i}"][:])
                b2 = wpool.tile(bshape, F32, tag=f"b2_{i}")
                nc.sync.dma_start(out=b2[:], in_=wd[f"b2_{i}"][:])
                wsb[i] = (w1, w2, b1, b2)
            lnw_sb = wpool.tile([128, c.NOUT], F32, tag="lnw")
            nc.sync.dma_start(out=lnw_sb[:], in_=ln_w_d[:])
            lnb_sb = wpool.tile([128, c.NOUT], F32, tag="lnb")
            nc.sync.dma_start(out=lnb_sb[:], in_=ln_b_d[:])

            staging = dram.tile([c.SHP, c.DPAD], BF16)
            tbl = [dram.tile([c.NP, c.DPAD], BF16, tag=f"tbl{j}", name=f"tbl{j}") for j in range(2)]
            prered = dram.tile([c.G_CH * 128, c.NOUT + 1], F32)
            postred = dram.tile([c.G_CH * 128, c.NOUT + 1], F32)

            pool_ps = [ppsum.tile([128, c.NOUT + 1], F32, tag=f"pool{gc}", name=f"pool{gc}")
                       for gc in range(c.G_CH)]

            for i in range(c.L):
                d_in = c.F if i == 0 else c.HID
                ch_in = (d_in + 127) // 128
                d_out = c.NOUT if i == c.L - 1 else c.HID
                ch_h = (d_out + 127) // 128
                dop = ch_h * 128
                last = i == c.L - 1
                w1, w2, b1, b2 = wsb[i]
                gdt = F32 if i == 0 else BF16
                sdt = F32 if i == 0 else BF16
                iota_x = iota_f32 if i == 0 else iota_bf
                ident = id_f32 if i == 0 else id_bf
                gcols = c.F if i == 0 else c.DPAD
                table_t = x_table if i == 0 else tbl[(i - 1) % 2]

                for ws in c.WGS:
                    nw = len(ws)
                    NN = nw * 128
                    nt = nw * c.T
                    gbuf = gpool.tile([128, nt, gcols], gdt, tag="gbuf")
                    for wp, w in enumerate(ws):
                        for k in range(c.T):
                            gidx = w * c.T + k
                            nc.gpsimd.indirect_dma_start(
                                out=gbuf[:, wp * c.T + k, :], out_offset=None,
                                in_=table_t[:],
                                in_offset=bass.IndirectOffsetOnAxis(
                                    ap=idx_sb[:, gidx:gidx + 1], axis=0))
                    mT = [tpool.tile([128, NN], BF16, tag=f"mT{ci}", name=f"mT{ci}") for ci in range(ch_in)]
                    for wp, w in enumerate(ws):
                        aps = apsum.tile([128, d_in], F32, tag="agg")
                        for tn in range(c.T):
                            gidx = w * c.T + tn
                            S = spool.tile([128, 128], sdt, tag="S")
                            nc.vector.tensor_tensor(
                                out=S[:], in0=dsel_sb[:, gidx:gidx + 1].to_broadcast([128, 128]),
                                in1=iota_x[:], op=OP.is_equal)
                            nc.tensor.matmul(out=aps[:], lhsT=S[:],
                                             rhs=gbuf[:, wp * c.T + tn, 0:d_in],
                                             start=(tn == 0), stop=(tn == c.T - 1))
                        res = respool.tile([128, gcols], gdt, tag="res")
                        if i == 0:
                            nc.sync.dma_start(out=res[:], in_=x_local[w * 128:(w + 1) * 128, :])
                        else:
                            nc.sync.dma_start(out=res[:], in_=staging[w * 128:(w + 1) * 128, :])
                        m = mpool.tile([128, ch_in * 128], BF16 if i else F32, tag="m")
                        if d_in < ch_in * 128:
                            nc.vector.memset(m[:, d_in:ch_in * 128], 0.0)
                        nc.vector.tensor_add(out=m[:, 0:d_in], in0=res[:, 0:d_in], in1=aps[:])
                        for ci in range(ch_in):
                            tp = tpsum.tile([128, 128], F32 if i == 0 else BF16, tag="tp", name="tp")
                            nc.tensor.transpose(out=tp[:], in_=m[:, ci * 128:(ci + 1) * 128],
                                                identity=ident[:])
                            nc.vector.tensor_copy(out=mT[ci][:, wp * 128:(wp + 1) * 128], in_=tp[:])
                    if not last:
                        zT = [tpool.tile([128, NN], BF16, tag=f"zT{co}", name=f"zT{co}") for co in range(ch_h)]
                        for co in range(ch_h):
                            zp = mlpps.tile([128, 1024], F32, tag="mlp", name="zp")[:, 0:NN]
                            for ki in range(ch_in):
                                nc.tensor.matmul(out=zp, lhsT=w1[:, ki * dop + co * 128: ki * dop + (co + 1) * 128],
                                                 rhs=mT[ki][:], start=(ki == 0), stop=(ki == ch_in - 1))
                            nc.scalar.activation(out=zT[co][:], in_=zp, func=AF.Relu,
                                                 bias=b1[:, co:co + 1], scale=1.0)
                        hT = [tpool.tile([128, NN], BF16, tag=f"hT{co}", name=f"hT{co}") for co in range(ch_h)]
                        for co in range(ch_h):
                            hp = mlpps.tile([128, 1024], F32, tag="mlp", name="hp")[:, 0:NN]
                            for ki in range(ch_h):
                                nc.tensor.matmul(out=hp, lhsT=w2[:, ki * dop + co * 128: ki * dop + (co + 1) * 128],
                                                 rhs=zT[ki][:], start=(ki == 0), stop=(ki == ch_h - 1))
                            nc.scalar.activation(out=hT[co][:], in_=hp, func=AF.Relu,
                                                 bias=b2[:, co:co + 1], scale=1.0)
                        for wp, w in enumerate(ws):
                            hnext = mpool.tile([128, ch_h * 128], BF16, tag="hnext")
                            for ci in range(ch_h):
                                tb = tpsum.tile([128, 128], BF16, tag="tp", name="tb")
                                nc.tensor.transpose(out=tb[:], in_=hT[ci][:, wp * 128:(wp + 1) * 128],
                                                    identity=id_bf[:])
                                nc.vector.tensor_copy(out=hnext[:, ci * 128:(ci + 1) * 128], in_=tb[:])
                            nc.sync.dma_start(out=staging[w * 128:(w + 1) * 128, :], in_=hnext[:, 0:c.HID])
                    else:
                        for wp, w in enumerate(ws):
                            t5p = mlpps.tile([128, 1024], F32, tag="mlp", name="t5p")
                            for ki in range(ch_in):
                                for nh in range(c.NOUT // 384):
                                    nc.tensor.matmul(
                                        out=t5p[:, nh * 512: nh * 512 + 384],
                                        lhsT=mT[ki][:, wp * 128:(wp + 1) * 128],
                                        rhs=w1[:, ki * dop + nh * 384: ki * dop + (nh + 1) * 384],
                                        start=(ki == 0), stop=(ki == ch_in - 1))
                            t5 = mpool.tile([128, c.NOUT + 1], BF16, tag="t5")
                            for nh in range(c.NOUT // 384):
                                nc.vector.tensor_add(out=t5[:, nh * 384:(nh + 1) * 384],
                                                     in0=t5p[:, nh * 512: nh * 512 + 384],
                                                     in1=b1[:, nh * 384:(nh + 1) * 384])
                            nc.vector.tensor_scalar_max(out=t5[:, 0:c.NOUT], in0=t5[:, 0:c.NOUT], scalar1=0.0)
                            nc.vector.tensor_copy(out=t5[:, c.NOUT:c.NOUT + 1], in_=mask_sb[:, w:w + 1])
                            for gc in range(c.G_CH):
                                SP = spool.tile([128, 128], BF16, tag="SP")
                                nc.vector.tensor_tensor(
                                    out=SP[:], in0=bg_sb[:, gc * c.WPC + w: gc * c.WPC + w + 1].to_broadcast([128, 128]),
                                    in1=iota_bf[:], op=OP.is_equal)
                                nc.tensor.matmul(out=pool_ps[gc][:, 0:512], lhsT=SP[:], rhs=t5[:, 0:512],
                                                 start=(w == 0), stop=(w == c.WPC - 1))
                                nc.tensor.matmul(out=pool_ps[gc][:, 512:c.NOUT + 1], lhsT=SP[:],
                                                 rhs=t5[:, 512:c.NOUT + 1],
                                                 start=(w == 0), stop=(w == c.WPC - 1))

                if not last:
                    nxt = tbl[i % 2]
                    nc.gpsimd.collective_compute(
                        "AllGather", OP.bypass, replica_groups=RG,
                        ins=[staging[:].opt()], outs=[nxt[:].opt()])

            NO1 = c.NOUT + 1
            psb = fpool.tile([128, c.G_CH * NO1], F32, tag="psb")
            for gc in range(c.G_CH):
                nc.vector.tensor_copy(out=psb[:, gc * NO1:(gc + 1) * NO1], in_=pool_ps[gc][:])
                nc.sync.dma_start(out=prered[gc * 128:(gc + 1) * 128, :],
                                  in_=psb[:, gc * NO1:(gc + 1) * NO1])
            nc.gpsimd.collective_compute(
                "AllReduce", OP.add, replica_groups=RG,
                ins=[prered[:].opt()], outs=[postred[:].opt()])

            w1_4, w2_4, b1_4, b2_4 = wsb[c.L - 1]
            meanT = [fpool.tile([128, c.G_CH * 128], BF16, tag=f"meanT{ci}", name=f"meanT{ci}") for ci in range(c.CHO)]
            for gc in range(c.G_CH):
                red = fpool.tile([128, NO1], F32, tag="red")
                nc.sync.dma_start(out=red[:], in_=postred[gc * 128:(gc + 1) * 128, :])
                cnt = fpool.tile([128, 1], F32, tag="cnt")
                nc.vector.tensor_scalar_max(out=cnt[:], in0=red[:, c.NOUT:NO1], scalar1=1.0)
                rec = fpool.tile([128, 1], F32, tag="rec")
                nc.vector.reciprocal(out=rec[:], in_=cnt[:])
                mean = fpool.tile([128, c.NOUT], F32, tag="mean")
                nc.vector.tensor_mul(out=mean[:], in0=red[:, 0:c.NOUT],
                                     in1=rec[:].to_broadcast([128, c.NOUT]))
                for ci in range(c.CHO):
                    tpm = tpsum.tile([128, 128], F32, tag="tp", name="tpm")
                    nc.tensor.transpose(out=tpm[:], in_=mean[:, ci * 128:(ci + 1) * 128],
                                        identity=id_f32[:])
                    nc.vector.tensor_copy(out=meanT[ci][:, gc * 128:(gc + 1) * 128], in_=tpm[:])
            for gc in range(c.G_CH):
                fp = mlpps.tile([128, 1024], F32, tag="mlp", name="fp")
                for ci in range(c.CHO):
                    for nh in range(c.NOUT // 384):
                        nc.tensor.matmul(
                            out=fp[:, nh * 512: nh * 512 + 384],
                            lhsT=meanT[ci][:, gc * 128:(gc + 1) * 128],
                            rhs=w2_4[:, ci * c.NOUT + nh * 384: ci * c.NOUT + (nh + 1) * 384],
                            start=(ci == 0), stop=(ci == c.CHO - 1))
                fin = fpool.tile([128, c.NOUT], F32, tag="fin")
                for nh in range(c.NOUT // 384):
                    nc.vector.tensor_add(out=fin[:, nh * 384:(nh + 1) * 384],
                                         in0=fp[:, nh * 512: nh * 512 + 384],
                                         in1=b2_4[:, nh * 384:(nh + 1) * 384])
                mu = fpool.tile([128, 1], F32, tag="mu")
                nc.vector.reduce_sum(out=mu[:], in_=fin[:], axis=mybir.AxisListType.X)
                nc.vector.tensor_scalar_mul(out=mu[:], in0=mu[:], scalar1=1.0 / c.NOUT)
                xc = fpool.tile([128, c.NOUT], F32, tag="xc")
                nc.vector.tensor_sub(out=xc[:], in0=fin[:], in1=mu[:].to_broadcast([128, c.NOUT]))
                sq = fpool.tile([128, c.NOUT], F32, tag="sq")
                nc.vector.tensor_mul(out=sq[:], in0=xc[:], in1=xc[:])
                vs = fpool.tile([128, 1], F32, tag="vs")
                nc.vector.reduce_sum(out=vs[:], in_=sq[:], axis=mybir.AxisListType.X)
                sd = fpool.tile([128, 1], F32, tag="sd")
                nc.scalar.activation(out=sd[:], in_=vs[:], func=AF.Sqrt,
                                     bias=eps_sb[:, 0:1], scale=1.0 / c.NOUT)
                rs = fpool.tile([128, 1], F32, tag="rs")
                nc.vector.reciprocal(out=rs[:], in_=sd[:])
                on = fpool.tile([128, c.NOUT], F32, tag="on")
                nc.vector.tensor_mul(out=on[:], in0=xc[:], in1=rs[:].to_broadcast([128, c.NOUT]))
                nc.vector.tensor_mul(out=on[:], in0=on[:], in1=lnw_sb[:])
                nc.vector.tensor_add(out=on[:], in0=on[:], in1=lnb_sb[:])
                rows = min(128, c.G - gc * 128)
                nc.sync.dma_start(out=out_d[gc * 128: gc * 128 + rows, :], in_=on[0:rows, :])

    nc.compile()
    return nc


_CACHE = {}


def _get_nc(T):
    key = T
    if key not in _CACHE:
        _CACHE[key] = build(Cfg(N=50000, E=800000, G=256, T_half=T))
    return _CACHE[key]


def kernel(**inputs):
    from concourse import bass_utils
    x = np.asarray(inputs["x"], np.float32)
    edge_index = np.asarray(inputs["edge_index"])
    batch = np.asarray(inputs["batch"])
    params = inputs["params"]
    T = 18
    cfg = Cfg(N=50000, E=800000, G=256, T_half=T)
    try:
        in_maps, maxcnt = prep_inputs(x, edge_index, batch, params, cfg)
    except RuntimeError:
        # extremely unbalanced window: rebuild with enough tiles
        deg = np.bincount(np.asarray(edge_index[1], np.int64), minlength=cfg.N)
        T = int(np.ceil((deg.max() * 128 + 50000) / 128.0)) + 20  # generous
        cfg = Cfg(N=50000, E=800000, G=256, T_half=T)
        in_maps, maxcnt = prep_inputs(x, edge_index, batch, params, cfg)
    nc = _get_nc(T)
    res = bass_utils.run_bass_kernel_spmd(nc, in_maps, core_ids=list(range(cfg.NC)))
    return np.ascontiguousarray(res.results[0]["out"].astype(np.float32))


# revision 2
# speedup vs baseline: 1.0769x; 1.0769x over previous
"""nn_BaseGraphEncoder GIN message-passing kernel for 8 Trainium2 NeuronCores.

Self-contained: host-side sharding prep (numpy) + Bass/Tile device kernel.
See _DOC below for the strategy summary.
"""
_DOC = """
  - dst-shard nodes across 8 cores (6250/core, padded to 6272 = 49 windows x 128,
    windows balanced by in-degree).
  - per-layer bf16 node-feature table [50176, 300] replicated in each core's DRAM
    via AllGather; per-edge gather via indirect DMA (128 rows/call, int32 ids).
  - aggregation: per 128-edge tile, one-hot dst matrix (DVE is_equal vs iota),
    PE matmul S^T @ msgs accumulated in PSUM per 128-dst window.
  - MLP: feature-major (transposed) activations, weight-stationary matmuls,
    fp32 PSUM, ACT fused bias+relu; PE transposes at orientation flips.
  - layer 4: pooling pushed before W2 (segsum(h5) = segsum(t5) @ W2 + cnt*b2);
    one-hot pooling matmuls with appended mask column for counts; AllReduce of
    [256,769] partials; fold W2; LayerNorm; identical output on every core.
"""
import sys

sys.path.insert(0, "/opt/trn_rl_repo")
import numpy as np
import concourse.bass as bass
import concourse.bacc as bacc
import concourse.tile as tile
import concourse.mybir as mybir

F32 = mybir.dt.float32
BF16 = mybir.dt.bfloat16
I32 = mybir.dt.int32
AF = mybir.ActivationFunctionType
OP = mybir.AluOpType
NPBF16 = mybir.dt.np(BF16)

LN_EPS = 1e-5


class Cfg:
    def __init__(self, N, E, G, T_half, F=128, HID=300, NOUT=768, NC=8, L=5):
        self.N, self.E, self.G, self.F, self.HID, self.NOUT, self.NC, self.L = (
            N, E, G, F, HID, NOUT, NC, L)
        assert N % NC == 0
        self.SH = N // NC
        self.WPC = (self.SH + 127) // 128
        self.SHP = self.WPC * 128
        self.NP = self.SHP * NC
        self.T = T_half                     # tiles per window
        self.HSW = (self.WPC // 2)             # windows in first staging half
        self.HS = self.HSW * 128               # rows in first half (per core)
        self.HS2 = self.SHP - self.HS
        self.DPAD = HID
        self.CH = (HID + 127) // 128
        self.CHO = NOUT // 128
        self.G_CH = (G + 127) // 128
        self.TILES = self.WPC * self.T
        self.WGS = [list(range(w, min(w + 2, self.WPC))) for w in range(0, self.WPC, 2)]


def prep_inputs(x, edge_index, batch, params, cfg):
    c = cfg
    x = np.asarray(x, np.float32)
    src = np.asarray(edge_index[0], np.int64)
    dst = np.asarray(edge_index[1], np.int64)
    batch = np.asarray(batch, np.int64)

    deg = np.bincount(dst, minlength=c.N)

    slot_of = np.empty(c.N, np.int64)
    import heapq
    for core in range(c.NC):
        nodes = np.arange(core * c.SH, (core + 1) * c.SH)
        order = nodes[np.argsort(-deg[nodes], kind="stable")]
        heap = [(0, 0, w) for w in range(c.WPC)]
        heapq.heapify(heap)
        fill = np.zeros(c.WPC, np.int64)
        for n in order:
            while True:
                load, cnt, w = heapq.heappop(heap)
                if fill[w] < 128:
                    break
            slot_of[n] = w * 128 + fill[w]
            fill[w] += 1
            heapq.heappush(heap, (load + deg[n], int(fill[w]), w))
    core_of = np.arange(c.N) // c.SH
    pgid = np.where(slot_of < c.HS,
                    core_of * c.HS + slot_of,
                    c.NC * c.HS + core_of * c.HS2 + (slot_of - c.HS))

    ecore = dst // c.SH
    ew = slot_of[dst] // 128
    epos = slot_of[dst] % 128
    esrc = pgid[src]

    idx_arr = np.zeros((c.NC, 128, c.TILES), np.int32)
    dsel_arr = np.full((c.NC, 128, c.TILES), 255.0, np.float32)

    maxcnt = 0
    for core in range(c.NC):
        em = ecore == core
        order = np.lexsort((epos[em], ew[em]))
        cw, cp, cs = ew[em][order], epos[em][order], esrc[em][order]
        for w in range(c.WPC):
            m = cw == w
            cnt = int(m.sum())
            maxcnt = max(maxcnt, cnt)
            if cnt > c.T * 128:
                raise RuntimeError(f"tile overflow: {cnt} > {c.T*128}")
            g0 = w * c.T
            j = np.arange(cnt)
            gt = g0 + j // 128
            lane = j % 128
            idx_arr[core, lane, gt] = cs[m]
            dsel_arr[core, lane, gt] = cp[m]

    x_table = np.zeros((c.NP, c.F), np.float32)
    x_table[pgid] = x
    x_local = np.zeros((c.NC, c.SHP, c.F), np.float32)
    for core in range(c.NC):
        nodes = np.arange(core * c.SH, (core + 1) * c.SH)
        x_local[core, slot_of[nodes]] = x[nodes]

    bgs = np.full((c.NC, c.G_CH, 128, c.WPC), 300.0, np.float32)
    maskw = np.zeros((c.NC, 128, c.WPC), np.float32)
    for core in range(c.NC):
        nodes = np.arange(core * c.SH, (core + 1) * c.SH)
        sl = slot_of[nodes]
        w, p = sl // 128, sl % 128
        maskw[core, p, w] = 1.0
        for gc in range(c.G_CH):
            val = batch[nodes] - gc * 128
            ok = (val >= 0) & (val < 128)
            bgs[core, gc, p[ok], w[ok]] = val[ok]

    def padw(W, rpad, cpad):
        W = np.asarray(W, np.float32)
        out = np.zeros((rpad, cpad), np.float32)
        out[: W.shape[0], : W.shape[1]] = W
        return out.astype(NPBF16)

    wts = {}
    layers = params["layers"]
    for i in range(c.L):
        d_in = c.F if i == 0 else c.HID
        d_out = c.NOUT if i == c.L - 1 else c.HID
        ch_in = (d_in + 127) // 128
        ch_h = (d_out + 127) // 128
        dop = ch_h * 128
        wts[f"W1_{i}"] = padw(layers[i]["W1"], ch_in * 128, dop)
        wts[f"W2_{i}"] = padw(layers[i]["W2"], ch_h * 128, dop)
        b1 = np.zeros(ch_h * 128, np.float32); b1[:d_out] = np.asarray(layers[i]["b1"])
        b2 = np.zeros(ch_h * 128, np.float32); b2[:d_out] = np.asarray(layers[i]["b2"])
        if i < c.L - 1:
            wts[f"b1_{i}"] = b1.reshape(ch_h, 128).T.copy()
            wts[f"b2_{i}"] = b2.reshape(ch_h, 128).T.copy()
        else:
            wts[f"b1_{i}"] = np.tile(b1[None, : c.NOUT], (128, 1))
            wts[f"b2_{i}"] = np.tile(b2[None, : c.NOUT], (128, 1))
    wts["ln_w"] = np.tile(np.asarray(params["ln_w"], np.float32)[None, :], (128, 1))
    wts["ln_b"] = np.tile(np.asarray(params["ln_b"], np.float32)[None, :], (128, 1))

    in_maps = []
    for core in range(c.NC):
        m = {
            "x_table": x_table,
            "x_local": x_local[core],
            "idxs": idx_arr[core],
            "dsel": dsel_arr[core].astype(NPBF16),
            "bg": bgs[core].reshape(c.G_CH * 128, c.WPC).astype(NPBF16),
            "maskw": maskw[core].astype(NPBF16),
        }
        m.update(wts)
        in_maps.append(m)
    return in_maps, maxcnt


def build(cfg):
    c = cfg
    nc = bacc.Bacc("TRN2", target_bir_lowering=False, debug=False, num_devices=c.NC)
    RG = [list(range(c.NC))]

    x_table = nc.dram_tensor("x_table", [c.NP, c.F], F32, kind="ExternalInput")
    x_local = nc.dram_tensor("x_local", [c.SHP, c.F], F32, kind="ExternalInput")
    idxs_d = nc.dram_tensor("idxs", [128, c.TILES], I32, kind="ExternalInput")
    dsel_d = nc.dram_tensor("dsel", [128, c.TILES], BF16, kind="ExternalInput")
    bg_d = nc.dram_tensor("bg", [c.G_CH * 128, c.WPC], BF16, kind="ExternalInput")
    maskw_d = nc.dram_tensor("maskw", [128, c.WPC], BF16, kind="ExternalInput")
    wd = {}
    for i in range(c.L):
        d_in = c.F if i == 0 else c.HID
        d_out = c.NOUT if i == c.L - 1 else c.HID
        ch_in = (d_in + 127) // 128
        ch_h = (d_out + 127) // 128
        dop = ch_h * 128
        wd[f"W1_{i}"] = nc.dram_tensor(f"W1_{i}", [ch_in * 128, dop], BF16, kind="ExternalInput")
        wd[f"W2_{i}"] = nc.dram_tensor(f"W2_{i}", [ch_h * 128, dop], BF16, kind="ExternalInput")
        bshape = [128, ch_h] if i < c.L - 1 else [128, c.NOUT]
        wd[f"b1_{i}"] = nc.dram_tensor(f"b1_{i}", bshape, F32, kind="ExternalInput")
        wd[f"b2_{i}"] = nc.dram_tensor(f"b2_{i}", bshape, F32, kind="ExternalInput")
    ln_w_d = nc.dram_tensor("ln_w", [128, c.NOUT], F32, kind="ExternalInput")
    ln_b_d = nc.dram_tensor("ln_b", [128, c.NOUT], F32, kind="ExternalInput")
    out_d = nc.dram_tensor("out", [c.G, c.NOUT], F32, kind="ExternalOutput")

    with tile.TileContext(nc) as tc:
        from concourse.masks import make_identity
        import contextlib
        st = contextlib.ExitStack()
        with st:
            const = st.enter_context(tc.tile_pool(name="const", bufs=1))
            wpool = st.enter_context(tc.tile_pool(name="wpool", bufs=1))
            edgep = st.enter_context(tc.tile_pool(name="edgep", bufs=1))
            dram = st.enter_context(tc.tile_pool(name="dram", bufs=1, space="DRAM"))
            gpool = st.enter_context(tc.tile_pool(name="gpool", bufs=4))
            spool = st.enter_context(tc.tile_pool(name="spool", bufs=6))
            mpool = st.enter_context(tc.tile_pool(name="mpool", bufs=3))
            tpool = st.enter_context(tc.tile_pool(name="tpool", bufs=2))
            apsum = st.enter_context(tc.tile_pool(name="apsum", bufs=1, space="PSUM"))
            tpsum = st.enter_context(tc.tile_pool(name="tpsum", bufs=1, space="PSUM"))
            mlpps = st.enter_context(tc.tile_pool(name="mlpps", bufs=1, space="PSUM"))
            ppsum = st.enter_context(tc.tile_pool(name="ppsum", bufs=1, space="PSUM"))
            respool = st.enter_context(tc.tile_pool(name="respool", bufs=3))
            fpool = st.enter_context(tc.tile_pool(name="fpool", bufs=1))

            id_f32 = const.tile([128, 128], F32)
            make_identity(nc, id_f32[:])
            id_bf = const.tile([128, 128], BF16)
            nc.vector.tensor_copy(out=id_bf[:], in_=id_f32[:])
            iota32 = const.tile([128, 128], I32)
            nc.gpsimd.iota(iota32[:], pattern=[[1, 128]], base=0, channel_multiplier=0)
            iota_bf = const.tile([128, 128], BF16)
            nc.vector.tensor_copy(out=iota_bf[:], in_=iota32[:])
            iota_f32 = const.tile([128, 128], F32)
            nc.vector.tensor_copy(out=iota_f32[:], in_=iota32[:])
            eps_sb = const.tile([128, 1], F32)
            nc.vector.memset(eps_sb[:], LN_EPS)

            idx_sb = edgep.tile([128, c.TILES], I32)
            nc.sync.dma_start(out=idx_sb[:], in_=idxs_d[:])
            dsel_sb = edgep.tile([128, c.TILES], BF16)
            nc.sync.dma_start(out=dsel_sb[:], in_=dsel_d[:])
            bg_sb = edgep.tile([128, c.G_CH * c.WPC], BF16)
            for gc in range(c.G_CH):
                nc.sync.dma_start(out=bg_sb[:, gc * c.WPC:(gc + 1) * c.WPC],
                                  in_=bg_d[gc * 128:(gc + 1) * 128, :])
            mask_sb = edgep.tile([128, c.WPC], BF16)
            nc.sync.dma_start(out=mask_sb[:], in_=maskw_d[:])

            wsb = {}
            for i in range(c.L):
                d_in = c.F if i == 0 else c.HID
                d_out = c.NOUT if i == c.L - 1 else c.HID
                ch_in = (d_in + 127) // 128
                ch_h = (d_out + 127) // 128
                dop = ch_h * 128
                w1 = wpool.tile([128, ch_in * dop], BF16, tag=f"w1_{i}")
                for ki in range(ch_in):
                    nc.sync.dma_start(out=w1[:, ki * dop:(ki + 1) * dop],
                                      in_=wd[f"W1_{i}"][ki * 128:(ki + 1) * 128, :])
                w2 = wpool.tile([128, ch_h * dop], BF16, tag=f"w2_{i}")
                for ki in range(ch_h):
                    nc.sync.dma_start(out=w2[:, ki * dop:(ki + 1) * dop],
                                      in_=wd[f"W2_{i}"][ki * 128:(ki + 1) * 128, :])
                bshape = [128, ch_h] if i < c.L - 1 else [128, c.NOUT]
                b1 = wpool.tile(bshape, F32, tag=f"b1_{i}")
                nc.sync.dma_start(out=b1[:], in_=wd[f"b1_{i}"][:])
                b2 = wpool.tile(bshape, F32, tag=f"b2_{i}")
                nc.sync.dma_start(out=b2[:], in_=wd[f"b2_{i}"][:])
                wsb[i] = (w1, w2, b1, b2)
            lnw_sb = wpool.tile([128, c.NOUT], F32, tag="lnw")
            nc.sync.dma_start(out=lnw_sb[:], in_=ln_w_d[:])
            lnb_sb = wpool.tile([128, c.NOUT], F32, tag="lnb")
            nc.sync.dma_start(out=lnb_sb[:], in_=ln_b_d[:])

            staging = dram.tile([c.SHP, c.DPAD], BF16)
            tbl = [dram.tile([c.NP, c.DPAD], BF16, tag=f"tbl{j}", name=f"tbl{j}") for j in range(2)]
            prered = dram.tile([c.G_CH * 128, c.NOUT + 1], F32)
            postred = dram.tile([c.G_CH * 128, c.NOUT + 1], F32)

            pool_ps = [ppsum.tile([128, c.NOUT + 1], F32, tag=f"pool{gc}", name=f"pool{gc}")
                       for gc in range(c.G_CH)]

            for i in range(c.L):
                d_in = c.F if i == 0 else c.HID
                ch_in = (d_in + 127) // 128
                d_out = c.NOUT if i == c.L - 1 else c.HID
                ch_h = (d_out + 127) // 128
                dop = ch_h * 128
                last = i == c.L - 1
                w1, w2, b1, b2 = wsb[i]
                gdt = F32 if i == 0 else BF16
                sdt = F32 if i == 0 else BF16
                iota_x = iota_f32 if i == 0 else iota_bf
                ident = id_f32 if i == 0 else id_bf
                gcols = c.F if i == 0 else c.DPAD
                table_t = x_table if i == 0 else tbl[(i - 1) % 2]

                for ws in c.WGS:
                    nw = len(ws)
                    NN = nw * 128
                    nt = nw * c.T
                    gbuf = gpool.tile([128, nt, gcols], gdt, tag="gbuf")
                    for wp, w in enumerate(ws):
                        for k in range(c.T):
                            gidx = w * c.T + k
                            nc.gpsimd.indirect_dma_start(
                                out=gbuf[:, wp * c.T + k, :], out_offset=None,
                                in_=table_t[:],
                                in_offset=bass.IndirectOffsetOnAxis(
                                    ap=idx_sb[:, gidx:gidx + 1], axis=0))
                    mT = [tpool.tile([128, NN], BF16, tag=f"mT{ci}", name=f"mT{ci}") for ci in range(ch_in)]
                    for wp, w in enumerate(ws):
                        aps = apsum.tile([128, d_in], F32, tag="agg")
                        for tn in range(c.T):
                            gidx = w * c.T + tn
                            S = spool.tile([128, 128], sdt, tag="S")
                            nc.vector.tensor_tensor(
                                out=S[:], in0=dsel_sb[:, gidx:gidx + 1].to_broadcast([128, 128]),
                                in1=iota_x[:], op=OP.is_equal)
                            nc.tensor.matmul(out=aps[:], lhsT=S[:],
                                             rhs=gbuf[:, wp * c.T + tn, 0:d_in],
                                             start=(tn == 0), stop=(tn == c.T - 1))
                        res = respool.tile([128, gcols], gdt, tag="res")
                        if i == 0:
                            nc.sync.dma_start(out=res[:], in_=x_local[w * 128:(w + 1) * 128, :])
                        else:
                            nc.sync.dma_start(out=res[:], in_=staging[w * 128:(w + 1) * 128, :])
                        m = mpool.tile([128, ch_in * 128], BF16 if i else F32, tag="m")
                        if d_in < ch_in * 128:
                            nc.vector.memset(m[:, d_in:ch_in * 128], 0.0)
                        nc.vector.tensor_add(out=m[:, 0:d_in], in0=res[:, 0:d_in], in1=aps[:])
                        for ci in range(ch_in):
                            tp = tpsum.tile([128, 128], F32 if i == 0 else BF16, tag="tp", name="tp")
                            nc.tensor.transpose(out=tp[:], in_=m[:, ci * 128:(ci + 1) * 128],
                                                identity=ident[:])
                            nc.vector.tensor_copy(out=mT[ci][:, wp * 128:(wp + 1) * 128], in_=tp[:])
                    if not last:
                        zT = [tpool.tile([128, NN], BF16, tag=f"zT{co}", name=f"zT{co}") for co in range(ch_h)]
                        for co in range(ch_h):
                            zp = mlpps.tile([128, 1024], F32, tag="mlp", name="zp")[:, 0:NN]
                            for ki in range(ch_in):
                                nc.tensor.matmul(out=zp, lhsT=w1[:, ki * dop + co * 128: ki * dop + (co + 1) * 128],
                                                 rhs=mT[ki][:], start=(ki == 0), stop=(ki == ch_in - 1))
                            nc.scalar.activation(out=zT[co][:], in_=zp, func=AF.Relu,
                                                 bias=b1[:, co:co + 1], scale=1.0)
                        hT = [tpool.tile([128, NN], BF16, tag=f"hT{co}", name=f"hT{co}") for co in range(ch_h)]
                        for co in range(ch_h):
                            hp = mlpps.tile([128, 1024], F32, tag="mlp", name="hp")[:, 0:NN]
                            for ki in range(ch_h):
                                nc.tensor.matmul(out=hp, lhsT=w2[:, ki * dop + co * 128: ki * dop + (co + 1) * 128],
                                                 rhs=zT[ki][:], start=(ki == 0), stop=(ki == ch_h - 1))
                            nc.scalar.activation(out=hT[co][:], in_=hp, func=AF.Relu,
                                                 bias=b2[:, co:co + 1], scale=1.0)
                        for wp, w in enumerate(ws):
                            hnext = mpool.tile([128, ch_h * 128], BF16, tag="hnext")
                            for ci in range(ch_h):
                                tb = tpsum.tile([128, 128], BF16, tag="tp", name="tb")
                                nc.tensor.transpose(out=tb[:], in_=hT[ci][:, wp * 128:(wp + 1) * 128],
                                                    identity=id_bf[:])
                                nc.vector.tensor_copy(out=hnext[:, ci * 128:(ci + 1) * 128], in_=tb[:])
                            nc.sync.dma_start(out=staging[w * 128:(w + 1) * 128, :], in_=hnext[:, 0:c.HID])
                        if (not last) and ws[-1] == c.HSW - 1 and wp == nw - 1:
                            nc.gpsimd.collective_compute(
                                "AllGather", OP.bypass, replica_groups=RG,
                                ins=[staging[0:c.HS, :].opt()],
                                outs=[tbl[i % 2][0:c.NC * c.HS, :].opt()])
                    else:
                        for wp, w in enumerate(ws):
                            t5p = mlpps.tile([128, 1024], F32, tag="mlp", name="t5p")
                            for ki in range(ch_in):
                                for nh in range(c.NOUT // 384):
                                    nc.tensor.matmul(
                                        out=t5p[:, nh * 512: nh * 512 + 384],
                                        lhsT=mT[ki][:, wp * 128:(wp + 1) * 128],
                                        rhs=w1[:, ki * dop + nh * 384: ki * dop + (nh + 1) * 384],
                                        start=(ki == 0), stop=(ki == ch_in - 1))
                            t5 = mpool.tile([128, c.NOUT + 1], BF16, tag="t5")
                            for nh in range(c.NOUT // 384):
                                nc.vector.tensor_add(out=t5[:, nh * 384:(nh + 1) * 384],
                                                     in0=t5p[:, nh * 512: nh * 512 + 384],
                                                     in1=b1[:, nh * 384:(nh + 1) * 384])
                            nc.vector.tensor_scalar_max(out=t5[:, 0:c.NOUT], in0=t5[:, 0:c.NOUT], scalar1=0.0)
                            nc.vector.tensor_copy(out=t5[:, c.NOUT:c.NOUT + 1], in_=mask_sb[:, w:w + 1])
                            for gc in range(c.G_CH):
                                SP = spool.tile([128, 128], BF16, tag="SP")
                                nc.vector.tensor_tensor(
                                    out=SP[:], in0=bg_sb[:, gc * c.WPC + w: gc * c.WPC + w + 1].to_broadcast([128, 128]),
                                    in1=iota_bf[:], op=OP.is_equal)
                                nc.tensor.matmul(out=pool_ps[gc][:, 0:512], lhsT=SP[:], rhs=t5[:, 0:512],
                                                 start=(w == 0), stop=(w == c.WPC - 1))
                                nc.tensor.matmul(out=pool_ps[gc][:, 512:c.NOUT + 1], lhsT=SP[:],
                                                 rhs=t5[:, 512:c.NOUT + 1],
                                                 start=(w == 0), stop=(w == c.WPC - 1))

                if not last:
                    nxt = tbl[i % 2]
                    nc.gpsimd.collective_compute(
                        "AllGather", OP.bypass, replica_groups=RG,
                        ins=[staging[c.HS:c.SHP, :].opt()],
                        outs=[nxt[c.NC * c.HS:c.NP, :].opt()])

            NO1 = c.NOUT + 1
            psb = fpool.tile([128, c.G_CH * NO1], F32, tag="psb")
            for gc in range(c.G_CH):
                nc.vector.tensor_copy(out=psb[:, gc * NO1:(gc + 1) * NO1], in_=pool_ps[gc][:])
                nc.sync.dma_start(out=prered[gc * 128:(gc + 1) * 128, :],
                                  in_=psb[:, gc * NO1:(gc + 1) * NO1])
            nc.gpsimd.collective_compute(
                "AllReduce", OP.add, replica_groups=RG,
                ins=[prered[:].opt()], outs=[postred[:].opt()])

            w1_4, w2_4, b1_4, b2_4 = wsb[c.L - 1]
            meanT = [fpool.tile([128, c.G_CH * 128], BF16, tag=f"meanT{ci}", name=f"meanT{ci}") for ci in range(c.CHO)]
            for gc in range(c.G_CH):
                red = fpool.tile([128, NO1], F32, tag="red")
                nc.sync.dma_start(out=red[:], in_=postred[gc * 128:(gc + 1) * 128, :])
                cnt = fpool.tile([128, 1], F32, tag="cnt")
                nc.vector.tensor_scalar_max(out=cnt[:], in0=red[:, c.NOUT:NO1], scalar1=1.0)
                rec = fpool.tile([128, 1], F32, tag="rec")
                nc.vector.reciprocal(out=rec[:], in_=cnt[:])
                mean = fpool.tile([128, c.NOUT], F32, tag="mean")
                nc.vector.tensor_mul(out=mean[:], in0=red[:, 0:c.NOUT],
                                     in1=rec[:].to_broadcast([128, c.NOUT]))
                for ci in range(c.CHO):
                    tpm = tpsum.tile([128, 128], F32, tag="tp", name="tpm")
                    nc.tensor.transpose(out=tpm[:], in_=mean[:, ci * 128:(ci + 1) * 128],
                                        identity=id_f32[:])
                    nc.vector.tensor_copy(out=meanT[ci][:, gc * 128:(gc + 1) * 128], in_=tpm[:])
            for gc in range(c.G_CH):
                fp = mlpps.tile([128, 1024], F32, tag="mlp", name="fp")
                for ci in range(c.CHO):
                    for nh in range(c.NOUT // 384):
                        nc.tensor.matmul(
                            out=fp[:, nh * 512: nh * 512 + 384],
                            lhsT=meanT[ci][:, gc * 128:(gc + 1) * 128],
                            rhs=w2_4[:, ci * c.NOUT + nh * 384: ci * c.NOUT + (nh + 1) * 384],
                            start=(ci == 0), stop=(ci == c.CHO - 1))
                fin = fpool.tile([128, c.NOUT], F32, tag="fin")
                for nh in range(c.NOUT // 384):
                    nc.vector.tensor_add(out=fin[:, nh * 384:(nh + 1) * 384],
                                         in0=fp[:, nh * 512: nh * 512 + 384],
                                         in1=b2_4[:, nh * 384:(nh + 1) * 384])
                mu = fpool.tile([128, 1], F32, tag="mu")
                nc.vector.reduce_sum(out=mu[:], in_=fin[:], axis=mybir.AxisListType.X)
                nc.vector.tensor_scalar_mul(out=mu[:], in0=mu[:], scalar1=1.0 / c.NOUT)
                xc = fpool.tile([128, c.NOUT], F32, tag="xc")
                nc.vector.tensor_sub(out=xc[:], in0=fin[:], in1=mu[:].to_broadcast([128, c.NOUT]))
                sq = fpool.tile([128, c.NOUT], F32, tag="sq")
                nc.vector.tensor_mul(out=sq[:], in0=xc[:], in1=xc[:])
                vs = fpool.tile([128, 1], F32, tag="vs")
                nc.vector.reduce_sum(out=vs[:], in_=sq[:], axis=mybir.AxisListType.X)
                sd = fpool.tile([128, 1], F32, tag="sd")
                nc.scalar.activation(out=sd[:], in_=vs[:], func=AF.Sqrt,
                                     bias=eps_sb[:, 0:1], scale=1.0 / c.NOUT)
                rs = fpool.tile([128, 1], F32, tag="rs")
                nc.vector.reciprocal(out=rs[:], in_=sd[:])
                on = fpool.tile([128, c.NOUT], F32, tag="on")
                nc.vector.tensor_mul(out=on[:], in0=xc[:], in1=rs[:].to_broadcast([128, c.NOUT]))
                nc.vector.tensor_mul(out=on[:], in0=on[:], in1=lnw_sb[:])
                nc.vector.tensor_add(out=on[:], in0=on[:], in1=lnb_sb[:])
                rows = min(128, c.G - gc * 128)
                nc.sync.dma_start(out=out_d[gc * 128: gc * 128 + rows, :], in_=on[0:rows, :])

    nc.compile()
    return nc


_CACHE = {}


def _get_nc(T):
    key = T
    if key not in _CACHE:
        _CACHE[key] = build(Cfg(N=50000, E=800000, G=256, T_half=T))
    return _CACHE[key]


def kernel(**inputs):
    from concourse import bass_utils
    x = np.asarray(inputs["x"], np.float32)
    edge_index = np.asarray(inputs["edge_index"])
    batch = np.asarray(inputs["batch"])
    params = inputs["params"]
    T = 18
    cfg = Cfg(N=50000, E=800000, G=256, T_half=T)
    try:
        in_maps, maxcnt = prep_inputs(x, edge_index, batch, params, cfg)
    except RuntimeError:
        # extremely unbalanced window: rebuild with enough tiles
        deg = np.bincount(np.asarray(edge_index[1], np.int64), minlength=cfg.N)
        T = int(np.ceil((deg.max() * 128 + 50000) / 128.0)) + 20  # generous
        cfg = Cfg(N=50000, E=800000, G=256, T_half=T)
        in_maps, maxcnt = prep_inputs(x, edge_index, batch, params, cfg)
    nc = _get_nc(T)
    res = bass_utils.run_bass_kernel_spmd(nc, in_maps, core_ids=list(range(cfg.NC)))
    return np.ascontiguousarray(res.results[0]["out"].astype(np.float32))
